# revision 6
# baseline (speedup 1.0000x reference)
"""Trainium2 Bass kernel for nn_HKANGNN (hetero GraphConv + KAN head).

Math (only the email-node output path matters):
  e    = x_email @ w_email.T + b_email
  agg_se[n] = sum_{se edges -> n} (x_sender[src] @ w_sender.T + b_sender)
  agg_ue[n] = sum_{ue edges -> n} (x_url[src]    @ w_url.T    + b_url)
  out_e = agg_se @ w_rel_se.T + b_rel_se + agg_ue @ w_rel_ue.T + b_rel_ue
        + e @ (w_root_se + w_root_ue).T
  h = relu(out_e);  out = silu(h) @ base_w.T + einsum(b_splines(h), spline_w)

Device strategy (8 cores, email nodes sharded 12500/core, padded to 12800):
  * linearity lets the per-edge features be the RAW source features plus a
    count column; the tiny projection matrices fold into Mcomb on host.
  * segment-sum = one-hot matmuls accumulated in PSUM per 128-dst tile;
    per-edge rows fetched with dma_gather (3 source classes so int16 idx fit).
  * projection: out_e.T[h, n] accumulated in PSUM over 6 K-chunks of
    (Wrootsum@w_email).T as stationary weights (bf16).
  * KAN head: h>=0 and all bases vanish for h>=2.2, so with x = clamp(h,0,2.2)
    spline(h) == q0+q1 x+q2 x^2+q3 x^3 + sum_k W'_k relu(x-t_k)^3 (t_k=.2..1.8)
    -> 10 extra matmul K-chunks ([silu,1,x,x^2,x^3,R6^3..R10^3]) into a [2,512]
    PSUM per 512-node tile.
"""

import os
import numpy as np
import ml_dtypes

import concourse.bass as bass
import concourse.mybir as mybir
import concourse.tile as tile
from concourse import bacc
from concourse.bass_utils import run_bass_kernel_spmd

F32 = mybir.dt.float32
BF16 = mybir.dt.bfloat16
BF = ml_dtypes.bfloat16

N_CORES = 8
HID = 128
NE, NS, NU = 100000, 30000, 50000
NSH = NE // N_CORES          # 12500 real nodes per core
NP = 12800                   # padded (25 x 512 node tiles, 100 x 128 dst tiles)
NT128 = NP // 128            # 100 dst tiles
NT512 = NP // 512            # 25 node tiles
KIN = 768
NKC = KIN // 128             # 6 projection K-chunks
URL_SPLIT = 25600            # url class A rows [0,25600), B rows [25600,50000)
ELEM = 128                   # gather row: 128 bf16 = 256 B
CH_T = 10                    # dst tiles per gather chunk
N_CH = NT128 // CH_T         # 10 chunks
KNOTS = (0.2, 0.6, 1.0, 1.4, 1.8)
XCLAMP = 2.2

_LAST_RESULT = None
_CACHE = {}


# ----------------------------------------------------------------- host folds
def _head_weights(base_w, spline_w):
    """[128, 20] f32: lhsT ([d,2]) per head chunk, order
    [silu, ones, x, x^2, x^3, R(.2)^3, R(.6)^3, R(1.0)^3, R(1.4)^3, R(1.8)^3]."""
    c = np.array([1.0, -4.0, 6.0, -4.0, 1.0], np.float64)
    h = 0.4
    scale = 1.0 / (6.0 * h ** 3)
    O, D, B = spline_w.shape                      # [2, 128, 8]
    wp = np.zeros((O, D, 11), np.float64)         # W'[o,d,m], m=0..10
    for m in range(11):
        for j in range(5):
            b = m - j
            if 0 <= b < B:
                wp[:, :, m] += spline_w[:, :, b].astype(np.float64) * c[j] * scale
    t = np.arange(11) * h - 2.2                   # knot m at t_m
    q = np.zeros((4, O, D), np.float64)           # poly coeffs from m=0..5
    for m in range(6):
        q[0] += -t[m] ** 3 * wp[:, :, m]
        q[1] += 3 * t[m] ** 2 * wp[:, :, m]
        q[2] += -3 * t[m] * wp[:, :, m]
        q[3] += wp[:, :, m]
    head = np.zeros((D, 20), np.float64)
    head[:, 0:2] = base_w.T                       # silu chunk
    for j in range(4):                            # ones, x, x^2, x^3
        head[:, 2 * (1 + j):2 * (1 + j) + 2] = q[j].T
    for k in range(5):                            # relu^3 knots m=6..10
        head[:, 2 * (5 + k):2 * (5 + k) + 2] = wp[:, :, 6 + k].T
    return head.astype(np.float32)


def _fold_weights(inp):
    wrs = inp["w_root_se"] + inp["w_root_ue"]
    wbigT = (wrs @ inp["w_email"]).T.copy()                     # [768, 128]
    mcomb = np.zeros((12, 128), np.float32)
    mcomb[0] = inp["w_rel_se"] @ inp["w_sender"][:, 0]
    mcomb[1] = inp["w_rel_se"] @ inp["b_sender"]
    mcomb[2:10] = (inp["w_rel_ue"] @ inp["w_url"]).T
    mcomb[10] = inp["w_rel_ue"] @ inp["b_url"]
    mcomb[11] = inp["b_rel_se"] + inp["b_rel_ue"] + wrs @ inp["b_email"]
    head = _head_weights(inp["base_w"], inp["spline_w"])
    return wbigT, mcomb, head


def _wrap_idx16(flat):
    """int16 slot list -> [128, n/16] wrapped in 16 partitions, tiled to 128."""
    n = flat.shape[0]
    a = flat.astype(np.int16).reshape(n // 16, 16).T            # [16, n/16]
    return np.tile(a, (8, 1))


def _prep_edges(inp):
    """Per-core per-class slot arrays (idx into class tables + local dst)."""
    cls_edges = []
    # (src_in_class_table, dst_email) per class
    cls_edges.append((inp["se_src"], inp["se_dst"]))                     # S
    ua = inp["ue_src"] < URL_SPLIT
    cls_edges.append((inp["ue_src"][ua], inp["ue_dst"][ua]))             # A
    cls_edges.append((inp["ue_src"][~ua] - URL_SPLIT, inp["ue_dst"][~ua]))  # B
    zrow = (NS, URL_SPLIT, NU - URL_SPLIT)                               # zero-row ids
    percls = []
    for ci, (src, dst) in enumerate(cls_edges):
        per_core = []
        gmax = 1
        for c in range(N_CORES):
            sel = (dst >= c * NSH) & (dst < (c + 1) * NSH)
            s, d = src[sel], dst[sel] - c * NSH
            order = np.argsort(d, kind="stable")
            s, d = s[order], d[order]
            tlist = []
            for t in range(NT128):
                m = (d >= t * 128) & (d < (t + 1) * 128)
                tlist.append((s[m], d[m] - t * 128))
                gmax = max(gmax, (len(tlist[-1][0]) + 127) // 128)
            per_core.append(tlist)
        percls.append((per_core, gmax, zrow[ci]))
    out = []
    for per_core, gmax, zr in percls:
        idxs, dsts = [], []
        for c in range(N_CORES):
            slots = np.full((NT128, gmax * 128), zr, np.int32)
            dloc = np.zeros((NT128, gmax * 128), np.float32)
            for t, (s, d) in enumerate(per_core[c]):
                slots[t, : len(s)] = s
                dloc[t, : len(s)] = d
            # slot j within a group of 128 -> partition j%128; groups tile-major
            flat = slots.reshape(-1)                             # [100*gmax*128]
            idxs.append(_wrap_idx16(flat))
            dsts.append(dloc.reshape(NT128 * gmax, 128).T.copy())  # [128, 100*gmax]
        out.append((np.stack(idxs), np.stack(dsts), gmax))
    return out  # [(idx16 [8,128,*], dst [8,128,100*g], g)] * 3


# ----------------------------------------------------------------- device build
def _build(gS, gA, gB):
    nc = bacc.Bacc("TRN2", target_bir_lowering=False, debug=False,
                   num_devices=N_CORES)
    dt = lambda n, s, d, k: nc.dram_tensor(n, s, d, kind=k).ap()
    xT = dt("xT", [KIN, NP], BF16, "ExternalInput")
    tabS = dt("tabS", [NS + 1, ELEM], BF16, "ExternalInput")
    tabA = dt("tabA", [URL_SPLIT + 1, ELEM], BF16, "ExternalInput")
    tabB = dt("tabB", [NU - URL_SPLIT + 1, ELEM], BF16, "ExternalInput")
    idxS = dt("idxS", [128, NT128 * gS * 8], mybir.dt.int16, "ExternalInput")
    idxA = dt("idxA", [128, NT128 * gA * 8], mybir.dt.int16, "ExternalInput")
    idxB = dt("idxB", [128, NT128 * gB * 8], mybir.dt.int16, "ExternalInput")
    dstS = dt("dstS", [128, NT128 * gS], F32, "ExternalInput")
    dstA = dt("dstA", [128, NT128 * gA], F32, "ExternalInput")
    dstB = dt("dstB", [128, NT128 * gB], F32, "ExternalInput")
    wbigT = dt("wbigT", [KIN, HID], BF16, "ExternalInput")
    mcomb = dt("mcomb", [12, HID], BF16, "ExternalInput")
    whead = dt("whead", [HID, 20], F32, "ExternalInput")
    outT = dt("outT", [2, NP], F32, "ExternalOutput")

    with tile.TileContext(nc) as tc:
        import contextlib
        with contextlib.ExitStack() as ctx:
            persist = ctx.enter_context(tc.tile_pool(name="persist", bufs=1))
            gpool = ctx.enter_context(tc.tile_pool(name="gath", bufs=2))
            xpool = ctx.enter_context(tc.tile_pool(name="x", bufs=2))
            ew = ctx.enter_context(tc.tile_pool(name="ew", bufs=2))
            psum = ctx.enter_context(tc.tile_pool(name="ps", bufs=2, space="PSUM"))

            # ---- persistent small tensors
            iota = persist.tile([128, 128], F32)
            nc.gpsimd.iota(iota[:], pattern=[[1, 128]], base=0,
                           channel_multiplier=0,
                           allow_small_or_imprecise_dtypes=True)
            ones = persist.tile([128, 512], F32)
            nc.gpsimd.memset(ones[:], 1.0)
            gTs = persist.tile([2, NP], BF16)
            gTu = persist.tile([9, NP], BF16)
            ones2 = persist.tile([1, 512], BF16)
            nc.gpsimd.memset(ones2[:], 1.0)
            wb = persist.tile([128, NKC * HID], BF16)
            nc.sync.dma_start(
                out=wb[:].rearrange("p (c h) -> p c h", c=NKC),
                in_=wbigT.rearrange("(c p) h -> p c h", p=128))
            mcS = persist.tile([2, HID], BF16)
            nc.sync.dma_start(out=mcS[:], in_=mcomb[0:2, :])
            mcU = persist.tile([9, HID], BF16)
            nc.sync.dma_start(out=mcU[:], in_=mcomb[2:11, :])
            mcC = persist.tile([1, HID], BF16)
            nc.sync.dma_start(out=mcC[:], in_=mcomb[11:12, :])
            wh = persist.tile([HID, 20], F32)
            nc.sync.dma_start(out=wh[:], in_=whead[:])
            dS = persist.tile([128, NT128 * gS], F32)
            nc.sync.dma_start(out=dS[:], in_=dstS[:])
            dA = persist.tile([128, NT128 * gA], F32)
            nc.sync.dma_start(out=dA[:], in_=dstA[:])
            dB = persist.tile([128, NT128 * gB], F32)
            nc.sync.dma_start(out=dB[:], in_=dstB[:])

            # ---- phase A: gather + one-hot scatter matmuls
            for ch in range(N_CH):
                t0 = ch * CH_T
                cls = []
                for tab, idxd, g, dloc in ((tabS, idxS, gS, dS),
                                           (tabA, idxA, gA, dA),
                                           (tabB, idxB, gB, dB)):
                    nidx = CH_T * g * 128
                    isb = gpool.tile([128, nidx // 16], mybir.dt.int16,
                                     tag=f"i{g}_{id(idxd)}")
                    nc.sync.dma_start(
                        out=isb[:], in_=idxd[:, t0 * g * 8:(t0 + CH_T) * g * 8])
                    gt = gpool.tile([128, CH_T * g, ELEM], BF16,
                                    tag=f"g{id(idxd)}")
                    nc.gpsimd.dma_gather(
                        out_ap=gt[:], in_ap=tab[:], idxs_ap=isb[:],
                        num_idxs=nidx, num_idxs_reg=nidx, elem_size=ELEM,
                        single_packet=False)
                    cls.append((gt, g, dloc))
                for tl in range(CH_T):
                    t = t0 + tl
                    pS = psum.tile([2, 128], F32, space="PSUM", tag="pS")
                    pU = psum.tile([9, 128], F32, space="PSUM", tag="pU")
                    for ci, (gt, g, dloc) in enumerate(cls):
                        for gi in range(g):
                            col = t * g + gi
                            oh = ew.tile([128, 128], BF16, tag="oh")
                            nc.vector.tensor_scalar(
                                out=oh[:], in0=iota[:],
                                scalar1=dloc[:, col:col + 1], scalar2=None,
                                op0=mybir.AluOpType.is_equal)
                            lhs = gt[:, tl * g + gi, 0:2 if ci == 0 else 9]
                            if ci == 0:
                                nc.tensor.matmul(out=pS[:], lhsT=lhs, rhs=oh[:],
                                                 start=(gi == 0),
                                                 stop=(gi == g - 1))
                            else:
                                nc.tensor.matmul(out=pU[:], lhsT=lhs, rhs=oh[:],
                                                 start=(ci == 1 and gi == 0),
                                                 stop=(ci == 2 and gi == g - 1))
                    sl = slice(t * 128, (t + 1) * 128)
                    nc.scalar.copy(out=gTs[:, sl], in_=pS[:])
                    nc.scalar.copy(out=gTu[:, sl], in_=pU[:])

            # ---- phase B: projection + KAN head per 512-node tile
            for nt in range(NT512):
                ns = slice(nt * 512, (nt + 1) * 512)
                xs = xpool.tile([128, NKC * 512], BF16, tag="xs")
                nc.sync.dma_start(
                    out=xs[:].rearrange("p (c n) -> p c n", c=NKC),
                    in_=xT[:, ns].rearrange("(c p) n -> p c n", p=128))
                pP = psum.tile([128, 512], F32, space="PSUM", tag="pP")
                for k in range(NKC):
                    nc.tensor.matmul(
                        out=pP[:], lhsT=wb[:, k * HID:(k + 1) * HID],
                        rhs=xs[:, k * 512:(k + 1) * 512],
                        start=(k == 0), stop=False)
                nc.tensor.matmul(out=pP[:], lhsT=mcS[:], rhs=gTs[:, ns],
                                 start=False, stop=False)
                nc.tensor.matmul(out=pP[:], lhsT=mcU[:], rhs=gTu[:, ns],
                                 start=False, stop=False)
                nc.tensor.matmul(out=pP[:], lhsT=mcC[:], rhs=ones2[:],
                                 start=False, stop=True)

                xt = ew.tile([128, 512], F32, tag="xt")      # clamp(h,0,2.2)
                nc.vector.tensor_scalar(out=xt[:], in0=pP[:], scalar1=0.0,
                                        scalar2=XCLAMP,
                                        op0=mybir.AluOpType.max,
                                        op1=mybir.AluOpType.min)
                sil = ew.tile([128, 512], F32, tag="sil")
                nc.scalar.activation(sil[:], pP[:],
                                     mybir.ActivationFunctionType.Silu)
                rsil = ew.tile([128, 512], F32, tag="rsil")
                nc.gpsimd.tensor_scalar_max(rsil[:], sil[:], 0.0)
                x2 = ew.tile([128, 512], F32, tag="x2")
                nc.scalar.square(x2[:], xt[:])
                x3 = ew.tile([128, 512], F32, tag="x3")
                nc.vector.tensor_tensor(out=x3[:], in0=x2[:], in1=xt[:],
                                        op=mybir.AluOpType.mult)
                r3s = []
                for k, tk in enumerate(KNOTS):
                    r = ew.tile([128, 512], F32, tag="r")
                    nc.gpsimd.tensor_scalar(out=r[:], in0=xt[:], scalar1=tk,
                                            scalar2=0.0,
                                            op0=mybir.AluOpType.subtract,
                                            op1=mybir.AluOpType.max)
                    r2 = ew.tile([128, 512], F32, tag="rr")
                    nc.scalar.square(r2[:], r[:])
                    r3 = ew.tile([128, 512], F32, tag=f"rrr{k}")
                    nc.vector.tensor_tensor(out=r3[:], in0=r2[:], in1=r[:],
                                            op=mybir.AluOpType.mult)
                    r3s.append(r3)
                pO = psum.tile([2, 512], F32, space="PSUM", tag="pO")
                chunks = [rsil, ones, xt, x2, x3] + r3s
                for j, ck in enumerate(chunks):
                    nc.tensor.matmul(out=pO[:], lhsT=wh[:, 2 * j:2 * j + 2],
                                     rhs=ck[:], start=(j == 0), stop=(j == 9))
                ot = ew.tile([2, 512], F32, tag="ot")
                nc.vector.tensor_copy(out=ot[:], in_=pO[:])
                nc.sync.dma_start(out=outT[:, ns], in_=ot[:])

    nc.compile()
    return nc


# ----------------------------------------------------------------- entry point
def kernel(**inp):
    inp = {k: np.asarray(v) for k, v in inp.items()}
    wbigT, mcomb, head = _fold_weights(inp)
    eprep = _prep_edges(inp)
    (idxS, dstS, gS), (idxA, dstA, gA), (idxB, dstB, gB) = eprep

    key = (gS, gA, gB)
    if key not in _CACHE:
        _CACHE[key] = _build(gS, gA, gB)
    nc = _CACHE[key]

    tabS = np.zeros((NS + 1, ELEM), BF)
    tabS[:NS, 0] = inp["x_sender"][:, 0].astype(BF)
    tabS[:NS, 1] = 1
    tabA = np.zeros((URL_SPLIT + 1, ELEM), BF)
    tabA[:URL_SPLIT, 0:8] = inp["x_url"][:URL_SPLIT].astype(BF)
    tabA[:URL_SPLIT, 8] = 1
    tabB = np.zeros((NU - URL_SPLIT + 1, ELEM), BF)
    tabB[: NU - URL_SPLIT, 0:8] = inp["x_url"][URL_SPLIT:].astype(BF)
    tabB[: NU - URL_SPLIT, 8] = 1

    in_maps = []
    for c in range(N_CORES):
        xsh = np.zeros((KIN, NP), BF)
        xsh[:, :NSH] = inp["x_email"][c * NSH:(c + 1) * NSH].T.astype(BF)
        in_maps.append({
            "xT": xsh, "tabS": tabS, "tabA": tabA, "tabB": tabB,
            "idxS": idxS[c], "idxA": idxA[c], "idxB": idxB[c],
            "dstS": dstS[c], "dstA": dstA[c], "dstB": dstB[c],
            "wbigT": wbigT.astype(BF), "mcomb": mcomb.astype(BF), "whead": head,
        })

    global _LAST_RESULT
    trace = os.environ.get("KERNEL_TRACE", "0") == "1"
    res = run_bass_kernel_spmd(nc, in_maps, core_ids=list(range(N_CORES)),
                               trace=trace)
    _LAST_RESULT = res
    out = np.empty((NE, 2), np.float32)
    for c in range(N_CORES):
        out[c * NSH:(c + 1) * NSH] = res.results[c]["outT"][:, :NSH].T
    return out


# revision 7
# speedup vs baseline: 1.9466x; 1.9466x over previous
"""Trainium2 Bass kernel for nn_HKANGNN (hetero GraphConv + KAN head).

Math (only the email-node output path matters):
  e    = x_email @ w_email.T + b_email
  agg_se[n] = sum_{se edges -> n} (x_sender[src] @ w_sender.T + b_sender)
  agg_ue[n] = sum_{ue edges -> n} (x_url[src]    @ w_url.T    + b_url)
  out_e = agg_se @ w_rel_se.T + b_rel_se + agg_ue @ w_rel_ue.T + b_rel_ue
        + e @ (w_root_se + w_root_ue).T
  h = relu(out_e);  out = silu(h) @ base_w.T + einsum(b_splines(h), spline_w)

Device strategy (8 cores, email nodes sharded 12500/core, padded to 12800):
  * linearity lets the per-edge features be the RAW source features plus a
    count column; the tiny projection matrices fold into Mcomb on host.
  * segment-sum = one-hot matmuls accumulated in PSUM per 128-dst tile;
    per-edge rows fetched with dma_gather (3 source classes so int16 idx fit).
  * projection: out_e.T[h, n] accumulated in PSUM over 6 K-chunks of
    (Wrootsum@w_email).T as stationary weights (bf16).
  * KAN head: h>=0 and all bases vanish for h>=2.2, so with x = clamp(h,0,2.2)
    spline(h) == q0+q1 x+q2 x^2+q3 x^3 + sum_k W'_k relu(x-t_k)^3 (t_k=.2..1.8)
    -> 10 extra matmul K-chunks ([silu,1,x,x^2,x^3,R6^3..R10^3]) into a [2,512]
    PSUM per 512-node tile.
"""

import os
import numpy as np
import ml_dtypes

import concourse.bass as bass
import concourse.mybir as mybir
import concourse.tile as tile
from concourse import bacc
from concourse.bass_utils import run_bass_kernel_spmd

F32 = mybir.dt.float32
BF16 = mybir.dt.bfloat16
BF = ml_dtypes.bfloat16

N_CORES = 8
HID = 128
NE, NS, NU = 100000, 30000, 50000
NSH = NE // N_CORES          # 12500 real nodes per core
NP = 12800                   # padded (25 x 512 node tiles, 100 x 128 dst tiles)
NT128 = NP // 128            # 100 dst tiles
NT512 = NP // 512            # 25 node tiles
KIN = 768
NKC = KIN // 128             # 6 projection K-chunks
URL_SPLIT = 25600            # url class A rows [0,25600), B rows [25600,50000)
ELEM = 128                   # gather row: 128 bf16 = 256 B
CH_T = 20                    # dst tiles per gather chunk
N_CH = NT128 // CH_T         # 10 chunks
KNOTS = (0.2, 0.6, 1.0, 1.4, 1.8)
XCLAMP = 2.2

_LAST_RESULT = None
_CACHE = {}


# ----------------------------------------------------------------- host folds
def _head_weights(base_w, spline_w):
    """[128, 20] f32: lhsT ([d,2]) per head chunk, order
    [silu, ones, x, x^2, x^3, R(.2)^3, R(.6)^3, R(1.0)^3, R(1.4)^3, R(1.8)^3]."""
    c = np.array([1.0, -4.0, 6.0, -4.0, 1.0], np.float64)
    h = 0.4
    scale = 1.0 / (6.0 * h ** 3)
    O, D, B = spline_w.shape                      # [2, 128, 8]
    wp = np.zeros((O, D, 11), np.float64)         # W'[o,d,m], m=0..10
    for m in range(11):
        for j in range(5):
            b = m - j
            if 0 <= b < B:
                wp[:, :, m] += spline_w[:, :, b].astype(np.float64) * c[j] * scale
    t = np.arange(11) * h - 2.2                   # knot m at t_m
    q = np.zeros((4, O, D), np.float64)           # poly coeffs from m=0..5
    for m in range(6):
        q[0] += -t[m] ** 3 * wp[:, :, m]
        q[1] += 3 * t[m] ** 2 * wp[:, :, m]
        q[2] += -3 * t[m] * wp[:, :, m]
        q[3] += wp[:, :, m]
    head = np.zeros((D, 20), np.float64)
    head[:, 0:2] = base_w.T                       # silu chunk
    for j in range(4):                            # ones, x, x^2, x^3
        head[:, 2 * (1 + j):2 * (1 + j) + 2] = q[j].T
    for k in range(5):                            # relu^3 knots m=6..10
        head[:, 2 * (5 + k):2 * (5 + k) + 2] = wp[:, :, 6 + k].T
    return head.astype(np.float32)


def _fold_weights(inp):
    wrs = inp["w_root_se"] + inp["w_root_ue"]
    wbigT = (wrs @ inp["w_email"]).T.copy()                     # [768, 128]
    mcomb = np.zeros((12, 128), np.float32)
    mcomb[0] = inp["w_rel_se"] @ inp["w_sender"][:, 0]
    mcomb[1] = inp["w_rel_se"] @ inp["b_sender"]
    mcomb[2:10] = (inp["w_rel_ue"] @ inp["w_url"]).T
    mcomb[10] = inp["w_rel_ue"] @ inp["b_url"]
    mcomb[11] = inp["b_rel_se"] + inp["b_rel_ue"] + wrs @ inp["b_email"]
    head = _head_weights(inp["base_w"], inp["spline_w"])
    return wbigT, mcomb, head


def _wrap_idx16(flat):
    """int16 slot list -> [128, n/16] wrapped in 16 partitions, tiled to 128."""
    n = flat.shape[0]
    a = flat.astype(np.int16).reshape(n // 16, 16).T            # [16, n/16]
    return np.tile(a, (8, 1))


def _prep_edges(inp):
    """Per-core per-class slot arrays (idx into class tables + local dst)."""
    cls_edges = []
    # (src_in_class_table, dst_email) per class
    cls_edges.append((inp["se_src"], inp["se_dst"]))                     # S
    ua = inp["ue_src"] < URL_SPLIT
    cls_edges.append((inp["ue_src"][ua], inp["ue_dst"][ua]))             # A
    cls_edges.append((inp["ue_src"][~ua] - URL_SPLIT, inp["ue_dst"][~ua]))  # B
    zrow = (NS, URL_SPLIT, NU - URL_SPLIT)                               # zero-row ids
    percls = []
    for ci, (src, dst) in enumerate(cls_edges):
        per_core = []
        gmax = 1
        for c in range(N_CORES):
            sel = (dst >= c * NSH) & (dst < (c + 1) * NSH)
            s, d = src[sel], dst[sel] - c * NSH
            order = np.argsort(d, kind="stable")
            s, d = s[order], d[order]
            tlist = []
            for t in range(NT128):
                m = (d >= t * 128) & (d < (t + 1) * 128)
                tlist.append((s[m], d[m] - t * 128))
                gmax = max(gmax, (len(tlist[-1][0]) + 127) // 128)
            per_core.append(tlist)
        percls.append((per_core, gmax, zrow[ci]))
    out = []
    for per_core, gmax, zr in percls:
        idxs, dsts = [], []
        for c in range(N_CORES):
            slots = np.full((NT128, gmax * 128), zr, np.int32)
            dloc = np.zeros((NT128, gmax * 128), np.float32)
            for t, (s, d) in enumerate(per_core[c]):
                slots[t, : len(s)] = s
                dloc[t, : len(s)] = d
            # slot j within a group of 128 -> partition j%128; groups tile-major
            flat = slots.reshape(-1)                             # [100*gmax*128]
            idxs.append(_wrap_idx16(flat))
            dsts.append(dloc.reshape(NT128 * gmax, 128).T.copy())  # [128, 100*gmax]
        out.append((np.stack(idxs), np.stack(dsts), gmax))
    return out  # [(idx16 [8,128,*], dst [8,128,100*g], g)] * 3


# ----------------------------------------------------------------- device build
def _build(gS, gA, gB):
    nc = bacc.Bacc("TRN2", target_bir_lowering=False, debug=False,
                   num_devices=N_CORES)
    dt = lambda n, s, d, k: nc.dram_tensor(n, s, d, kind=k).ap()
    xT = dt("xT", [KIN, NP], BF16, "ExternalInput")
    tabS = dt("tabS", [NS + 1, ELEM], BF16, "ExternalInput")
    tabA = dt("tabA", [URL_SPLIT + 1, ELEM], BF16, "ExternalInput")
    tabB = dt("tabB", [NU - URL_SPLIT + 1, ELEM], BF16, "ExternalInput")
    idxS = dt("idxS", [128, NT128 * gS * 8], mybir.dt.int16, "ExternalInput")
    idxA = dt("idxA", [128, NT128 * gA * 8], mybir.dt.int16, "ExternalInput")
    idxB = dt("idxB", [128, NT128 * gB * 8], mybir.dt.int16, "ExternalInput")
    dstS = dt("dstS", [128, NT128 * gS], F32, "ExternalInput")
    dstA = dt("dstA", [128, NT128 * gA], F32, "ExternalInput")
    dstB = dt("dstB", [128, NT128 * gB], F32, "ExternalInput")
    wbigT = dt("wbigT", [KIN, HID], BF16, "ExternalInput")
    mcomb = dt("mcomb", [12, HID], BF16, "ExternalInput")
    whead = dt("whead", [HID, 20], F32, "ExternalInput")
    outT = dt("outT", [2, NP], F32, "ExternalOutput")

    with tile.TileContext(nc) as tc:
        import contextlib
        with contextlib.ExitStack() as ctx:
            persist = ctx.enter_context(tc.tile_pool(name="persist", bufs=1))
            gpool = ctx.enter_context(tc.tile_pool(name="gath", bufs=2))
            xpool = ctx.enter_context(tc.tile_pool(name="x", bufs=2))
            ew = ctx.enter_context(tc.tile_pool(name="ew", bufs=2))
            psum = ctx.enter_context(tc.tile_pool(name="ps", bufs=2, space="PSUM"))

            # ---- persistent small tensors
            iota = persist.tile([128, 128], F32)
            nc.gpsimd.iota(iota[:], pattern=[[1, 128]], base=0,
                           channel_multiplier=0,
                           allow_small_or_imprecise_dtypes=True)
            ones = persist.tile([128, 512], F32)
            nc.gpsimd.memset(ones[:], 1.0)
            gTs = persist.tile([2, NP], BF16)
            gTu = persist.tile([9, NP], BF16)
            ones2 = persist.tile([1, 512], BF16)
            nc.gpsimd.memset(ones2[:], 1.0)
            wb = persist.tile([128, NKC * HID], BF16)
            nc.sync.dma_start(
                out=wb[:].rearrange("p (c h) -> p c h", c=NKC),
                in_=wbigT.rearrange("(c p) h -> p c h", p=128))
            mcS = persist.tile([2, HID], BF16)
            nc.sync.dma_start(out=mcS[:], in_=mcomb[0:2, :])
            mcU = persist.tile([9, HID], BF16)
            nc.sync.dma_start(out=mcU[:], in_=mcomb[2:11, :])
            mcC = persist.tile([1, HID], BF16)
            nc.sync.dma_start(out=mcC[:], in_=mcomb[11:12, :])
            wh = persist.tile([HID, 20], F32)
            nc.sync.dma_start(out=wh[:], in_=whead[:])
            dS = persist.tile([128, NT128 * gS], F32)
            nc.sync.dma_start(out=dS[:], in_=dstS[:])
            dA = persist.tile([128, NT128 * gA], F32)
            nc.sync.dma_start(out=dA[:], in_=dstA[:])
            dB = persist.tile([128, NT128 * gB], F32)
            nc.sync.dma_start(out=dB[:], in_=dstB[:])

            # ---- phase A: gather + one-hot scatter matmuls
            for ch in range(N_CH):
                t0 = ch * CH_T
                cls = []
                for tab, idxd, g, dloc in ((tabS, idxS, gS, dS),
                                           (tabA, idxA, gA, dA),
                                           (tabB, idxB, gB, dB)):
                    nidx = CH_T * g * 128
                    isb = gpool.tile([128, nidx // 16], mybir.dt.int16,
                                     tag=f"i{g}_{id(idxd)}")
                    nc.sync.dma_start(
                        out=isb[:], in_=idxd[:, t0 * g * 8:(t0 + CH_T) * g * 8])
                    gt = gpool.tile([128, CH_T * g, ELEM], BF16,
                                    tag=f"g{id(idxd)}")
                    nc.gpsimd.dma_gather(
                        out_ap=gt[:], in_ap=tab[:], idxs_ap=isb[:],
                        num_idxs=nidx, num_idxs_reg=nidx, elem_size=ELEM,
                        single_packet=False)
                    cls.append((gt, g, dloc))
                for tl in range(CH_T):
                    t = t0 + tl
                    pS = psum.tile([2, 128], F32, space="PSUM", tag="pS")
                    pU = psum.tile([9, 128], F32, space="PSUM", tag="pU")
                    for ci, (gt, g, dloc) in enumerate(cls):
                        for gi in range(g):
                            col = t * g + gi
                            oh = ew.tile([128, 128], BF16, tag="oh")
                            nc.vector.tensor_scalar(
                                out=oh[:], in0=iota[:],
                                scalar1=dloc[:, col:col + 1], scalar2=None,
                                op0=mybir.AluOpType.is_equal)
                            lhs = gt[:, tl * g + gi, 0:2 if ci == 0 else 9]
                            if ci == 0:
                                nc.tensor.matmul(out=pS[:], lhsT=lhs, rhs=oh[:],
                                                 start=(gi == 0),
                                                 stop=(gi == g - 1))
                            else:
                                nc.tensor.matmul(out=pU[:], lhsT=lhs, rhs=oh[:],
                                                 start=(ci == 1 and gi == 0),
                                                 stop=(ci == 2 and gi == g - 1))
                    sl = slice(t * 128, (t + 1) * 128)
                    nc.scalar.copy(out=gTs[:, sl], in_=pS[:])
                    nc.scalar.copy(out=gTu[:, sl], in_=pU[:])

            # ---- phase B: projection + KAN head per 512-node tile
            for nt in range(NT512):
                ns = slice(nt * 512, (nt + 1) * 512)
                xs = xpool.tile([128, NKC * 512], BF16, tag="xs")
                nc.sync.dma_start(
                    out=xs[:].rearrange("p (c n) -> p c n", c=NKC),
                    in_=xT[:, ns].rearrange("(c p) n -> p c n", p=128))
                pP = psum.tile([128, 512], F32, space="PSUM", tag="pP")
                for k in range(NKC):
                    nc.tensor.matmul(
                        out=pP[:], lhsT=wb[:, k * HID:(k + 1) * HID],
                        rhs=xs[:, k * 512:(k + 1) * 512],
                        start=(k == 0), stop=False)
                nc.tensor.matmul(out=pP[:], lhsT=mcS[:], rhs=gTs[:, ns],
                                 start=False, stop=False)
                nc.tensor.matmul(out=pP[:], lhsT=mcU[:], rhs=gTu[:, ns],
                                 start=False, stop=False)
                nc.tensor.matmul(out=pP[:], lhsT=mcC[:], rhs=ones2[:],
                                 start=False, stop=True)

                xt = ew.tile([128, 512], F32, tag="xt")      # clamp(h,0,2.2)
                nc.vector.tensor_scalar(out=xt[:], in0=pP[:], scalar1=0.0,
                                        scalar2=XCLAMP,
                                        op0=mybir.AluOpType.max,
                                        op1=mybir.AluOpType.min)
                sil = ew.tile([128, 512], F32, tag="sil")
                nc.scalar.activation(sil[:], pP[:],
                                     mybir.ActivationFunctionType.Silu)
                rsil = ew.tile([128, 512], F32, tag="rsil")
                nc.vector.tensor_scalar_max(rsil[:], sil[:], 0.0)
                x2 = ew.tile([128, 512], F32, tag="x2")
                nc.scalar.square(x2[:], xt[:])
                x3 = ew.tile([128, 512], F32, tag="x3")
                nc.vector.tensor_tensor(out=x3[:], in0=x2[:], in1=xt[:],
                                        op=mybir.AluOpType.mult)
                r3s = []
                for k, tk in enumerate(KNOTS):
                    r = ew.tile([128, 512], F32, tag="r")
                    nc.vector.tensor_scalar(out=r[:], in0=xt[:], scalar1=tk,
                                            scalar2=0.0,
                                            op0=mybir.AluOpType.subtract,
                                            op1=mybir.AluOpType.max)
                    r2 = ew.tile([128, 512], F32, tag="rr")
                    nc.scalar.square(r2[:], r[:])
                    r3 = ew.tile([128, 512], F32, tag=f"rrr{k}")
                    nc.vector.tensor_tensor(out=r3[:], in0=r2[:], in1=r[:],
                                            op=mybir.AluOpType.mult)
                    r3s.append(r3)
                pO = psum.tile([2, 512], F32, space="PSUM", tag="pO")
                chunks = [rsil, ones, xt, x2, x3] + r3s
                for j, ck in enumerate(chunks):
                    nc.tensor.matmul(out=pO[:], lhsT=wh[:, 2 * j:2 * j + 2],
                                     rhs=ck[:], start=(j == 0), stop=(j == 9))
                ot = ew.tile([2, 512], F32, tag="ot")
                nc.vector.tensor_copy(out=ot[:], in_=pO[:])
                nc.sync.dma_start(out=outT[:, ns], in_=ot[:])

    nc.compile()
    return nc


# ----------------------------------------------------------------- entry point
def kernel(**inp):
    inp = {k: np.asarray(v) for k, v in inp.items()}
    wbigT, mcomb, head = _fold_weights(inp)
    eprep = _prep_edges(inp)
    (idxS, dstS, gS), (idxA, dstA, gA), (idxB, dstB, gB) = eprep

    key = (gS, gA, gB)
    if key not in _CACHE:
        _CACHE[key] = _build(gS, gA, gB)
    nc = _CACHE[key]

    tabS = np.zeros((NS + 1, ELEM), BF)
    tabS[:NS, 0] = inp["x_sender"][:, 0].astype(BF)
    tabS[:NS, 1] = 1
    tabA = np.zeros((URL_SPLIT + 1, ELEM), BF)
    tabA[:URL_SPLIT, 0:8] = inp["x_url"][:URL_SPLIT].astype(BF)
    tabA[:URL_SPLIT, 8] = 1
    tabB = np.zeros((NU - URL_SPLIT + 1, ELEM), BF)
    tabB[: NU - URL_SPLIT, 0:8] = inp["x_url"][URL_SPLIT:].astype(BF)
    tabB[: NU - URL_SPLIT, 8] = 1

    in_maps = []
    for c in range(N_CORES):
        xsh = np.zeros((KIN, NP), BF)
        xsh[:, :NSH] = inp["x_email"][c * NSH:(c + 1) * NSH].T.astype(BF)
        in_maps.append({
            "xT": xsh, "tabS": tabS, "tabA": tabA, "tabB": tabB,
            "idxS": idxS[c], "idxA": idxA[c], "idxB": idxB[c],
            "dstS": dstS[c], "dstA": dstA[c], "dstB": dstB[c],
            "wbigT": wbigT.astype(BF), "mcomb": mcomb.astype(BF), "whead": head,
        })

    global _LAST_RESULT
    trace = os.environ.get("KERNEL_TRACE", "0") == "1"
    res = run_bass_kernel_spmd(nc, in_maps, core_ids=list(range(N_CORES)),
                               trace=trace)
    _LAST_RESULT = res
    out = np.empty((NE, 2), np.float32)
    for c in range(N_CORES):
        out[c * NSH:(c + 1) * NSH] = res.results[c]["outT"][:, :NSH].T
    return out


# revision 8
# speedup vs baseline: 2.0151x; 1.0352x over previous
"""Trainium2 Bass kernel for nn_HKANGNN (hetero GraphConv + KAN head).

Math (only the email-node output path matters):
  e    = x_email @ w_email.T + b_email
  agg_se[n] = sum_{se edges -> n} (x_sender[src] @ w_sender.T + b_sender)
  agg_ue[n] = sum_{ue edges -> n} (x_url[src]    @ w_url.T    + b_url)
  out_e = agg_se @ w_rel_se.T + b_rel_se + agg_ue @ w_rel_ue.T + b_rel_ue
        + e @ (w_root_se + w_root_ue).T
  h = relu(out_e);  out = silu(h) @ base_w.T + einsum(b_splines(h), spline_w)

Device strategy (8 cores, email nodes sharded 12500/core, padded to 12800):
  * linearity lets the per-edge features be the RAW source features plus a
    count column; the tiny projection matrices fold into Mcomb on host.
  * segment-sum = one-hot matmuls accumulated in PSUM per 128-dst tile;
    per-edge rows fetched with dma_gather (3 source classes so int16 idx fit).
  * projection: out_e.T[h, n] accumulated in PSUM over 6 K-chunks of
    (Wrootsum@w_email).T as stationary weights (bf16).
  * KAN head: h>=0 and all bases vanish for h>=2.2, so with x = clamp(h,0,2.2)
    spline(h) == q0+q1 x+q2 x^2+q3 x^3 + sum_k W'_k relu(x-t_k)^3 (t_k=.2..1.8)
    -> 10 extra matmul K-chunks ([silu,1,x,x^2,x^3,R6^3..R10^3]) into a [2,512]
    PSUM per 512-node tile.
"""

import os
import numpy as np
import ml_dtypes

import concourse.bass as bass
import concourse.mybir as mybir
import concourse.tile as tile
from concourse import bacc
from concourse.bass_utils import run_bass_kernel_spmd

F32 = mybir.dt.float32
BF16 = mybir.dt.bfloat16
BF = ml_dtypes.bfloat16

N_CORES = 8
HID = 128
NE, NS, NU = 100000, 30000, 50000
NSH = NE // N_CORES          # 12500 real nodes per core
NP = 12800                   # padded (25 x 512 node tiles, 100 x 128 dst tiles)
NT128 = NP // 128            # 100 dst tiles
NT512 = NP // 512            # 25 node tiles
KIN = 768
NKC = KIN // 128             # 6 projection K-chunks
URL_SPLIT = 25600            # url class A rows [0,25600), B rows [25600,50000)
ELEM = 128                   # gather row: 128 bf16 = 256 B
CH_T = 10                    # dst tiles per gather chunk
N_CH = NT128 // CH_T         # 10 chunks
KNOTS = (0.2, 0.6, 1.0, 1.4, 1.8)
XCLAMP = 2.2

_LAST_RESULT = None
_CACHE = {}


# ----------------------------------------------------------------- host folds
def _head_weights(base_w, spline_w):
    """[128, 20] f32: lhsT ([d,2]) per head chunk, order
    [silu, ones, x, x^2, x^3, R(.2)^3, R(.6)^3, R(1.0)^3, R(1.4)^3, R(1.8)^3]."""
    c = np.array([1.0, -4.0, 6.0, -4.0, 1.0], np.float64)
    h = 0.4
    scale = 1.0 / (6.0 * h ** 3)
    O, D, B = spline_w.shape                      # [2, 128, 8]
    wp = np.zeros((O, D, 11), np.float64)         # W'[o,d,m], m=0..10
    for m in range(11):
        for j in range(5):
            b = m - j
            if 0 <= b < B:
                wp[:, :, m] += spline_w[:, :, b].astype(np.float64) * c[j] * scale
    t = np.arange(11) * h - 2.2                   # knot m at t_m
    q = np.zeros((4, O, D), np.float64)           # poly coeffs from m=0..5
    for m in range(6):
        q[0] += -t[m] ** 3 * wp[:, :, m]
        q[1] += 3 * t[m] ** 2 * wp[:, :, m]
        q[2] += -3 * t[m] * wp[:, :, m]
        q[3] += wp[:, :, m]
    head = np.zeros((D, 20), np.float64)
    head[:, 0:2] = base_w.T                       # silu chunk
    for j in range(4):                            # ones, x, x^2, x^3
        head[:, 2 * (1 + j):2 * (1 + j) + 2] = q[j].T
    for k in range(5):                            # relu^3 knots m=6..10
        head[:, 2 * (5 + k):2 * (5 + k) + 2] = wp[:, :, 6 + k].T
    return head.astype(np.float32)


def _fold_weights(inp):
    wrs = inp["w_root_se"] + inp["w_root_ue"]
    wbigT = (wrs @ inp["w_email"]).T.copy()                     # [768, 128]
    mcomb = np.zeros((12, 128), np.float32)
    mcomb[0] = inp["w_rel_se"] @ inp["w_sender"][:, 0]
    mcomb[1] = inp["w_rel_se"] @ inp["b_sender"]
    mcomb[2:10] = (inp["w_rel_ue"] @ inp["w_url"]).T
    mcomb[10] = inp["w_rel_ue"] @ inp["b_url"]
    mcomb[11] = inp["b_rel_se"] + inp["b_rel_ue"] + wrs @ inp["b_email"]
    head = _head_weights(inp["base_w"], inp["spline_w"])
    return wbigT, mcomb, head


def _wrap_idx16(flat):
    """int16 slot list -> [128, n/16] wrapped in 16 partitions, tiled to 128."""
    n = flat.shape[0]
    a = flat.astype(np.int16).reshape(n // 16, 16).T            # [16, n/16]
    return np.tile(a, (8, 1))


def _prep_edges(inp):
    """Per-core per-class slot arrays (idx into class tables + local dst)."""
    cls_edges = []
    # (src_in_class_table, dst_email) per class
    cls_edges.append((inp["se_src"], inp["se_dst"]))                     # S
    ua = inp["ue_src"] < URL_SPLIT
    cls_edges.append((inp["ue_src"][ua], inp["ue_dst"][ua]))             # A
    cls_edges.append((inp["ue_src"][~ua] - URL_SPLIT, inp["ue_dst"][~ua]))  # B
    zrow = (NS, URL_SPLIT, NU - URL_SPLIT)                               # zero-row ids
    percls = []
    for ci, (src, dst) in enumerate(cls_edges):
        per_core = []
        gmax = 1
        for c in range(N_CORES):
            sel = (dst >= c * NSH) & (dst < (c + 1) * NSH)
            s, d = src[sel], dst[sel] - c * NSH
            order = np.argsort(d, kind="stable")
            s, d = s[order], d[order]
            tlist = []
            for t in range(NT128):
                m = (d >= t * 128) & (d < (t + 1) * 128)
                tlist.append((s[m], d[m] - t * 128))
                gmax = max(gmax, (len(tlist[-1][0]) + 127) // 128)
            per_core.append(tlist)
        percls.append((per_core, gmax, zrow[ci]))
    out = []
    for per_core, gmax, zr in percls:
        idxs, dsts = [], []
        for c in range(N_CORES):
            slots = np.full((NT128, gmax * 128), zr, np.int32)
            dloc = np.zeros((NT128, gmax * 128), np.float32)
            for t, (s, d) in enumerate(per_core[c]):
                slots[t, : len(s)] = s
                dloc[t, : len(s)] = d
            # slot j within a group of 128 -> partition j%128; groups tile-major
            flat = slots.reshape(-1)                             # [100*gmax*128]
            idxs.append(_wrap_idx16(flat))
            oh = (dloc.reshape(NT128 * gmax, 128).T[:, :, None]
                  == np.arange(128)[None, None, :]).astype(BF)
            dsts.append(oh.reshape(128, NT128 * gmax * 128))   # [128, ngrp*128]
        out.append((np.stack(idxs), np.stack(dsts), gmax))
    return out  # [(idx16 [8,128,*], dst [8,128,100*g], g)] * 3


# ----------------------------------------------------------------- device build
def _build(gS, gA, gB):
    nc = bacc.Bacc("TRN2", target_bir_lowering=False, debug=False,
                   num_devices=N_CORES)
    dt = lambda n, s, d, k: nc.dram_tensor(n, s, d, kind=k).ap()
    xT = dt("xT", [KIN, NP], BF16, "ExternalInput")
    tabS = dt("tabS", [NS + 1, ELEM], BF16, "ExternalInput")
    tabA = dt("tabA", [URL_SPLIT + 1, ELEM], BF16, "ExternalInput")
    tabB = dt("tabB", [NU - URL_SPLIT + 1, ELEM], BF16, "ExternalInput")
    idxS = dt("idxS", [128, NT128 * gS * 8], mybir.dt.int16, "ExternalInput")
    idxA = dt("idxA", [128, NT128 * gA * 8], mybir.dt.int16, "ExternalInput")
    idxB = dt("idxB", [128, NT128 * gB * 8], mybir.dt.int16, "ExternalInput")
    ohS = dt("ohS", [128, NT128 * gS * 128], BF16, "ExternalInput")
    ohA = dt("ohA", [128, NT128 * gA * 128], BF16, "ExternalInput")
    ohB = dt("ohB", [128, NT128 * gB * 128], BF16, "ExternalInput")
    wbigT = dt("wbigT", [KIN, HID], BF16, "ExternalInput")
    mcomb = dt("mcomb", [12, HID], BF16, "ExternalInput")
    whead = dt("whead", [HID, 20], F32, "ExternalInput")
    outT = dt("outT", [2, NP], F32, "ExternalOutput")

    with tile.TileContext(nc) as tc:
        import contextlib
        with contextlib.ExitStack() as ctx:
            persist = ctx.enter_context(tc.tile_pool(name="persist", bufs=1))
            gpool = ctx.enter_context(tc.tile_pool(name="gath", bufs=2))
            xpool = ctx.enter_context(tc.tile_pool(name="x", bufs=2))
            ew = ctx.enter_context(tc.tile_pool(name="ew", bufs=2))
            psum = ctx.enter_context(tc.tile_pool(name="ps", bufs=2, space="PSUM"))

            # ---- persistent small tensors
            ones = persist.tile([128, 512], F32)
            nc.gpsimd.memset(ones[:], 1.0)
            gTs = persist.tile([2, NP], BF16)
            gTu = persist.tile([9, NP], BF16)
            ones2 = persist.tile([1, 512], BF16)
            nc.gpsimd.memset(ones2[:], 1.0)
            wb = persist.tile([128, NKC * HID], BF16)
            nc.sync.dma_start(
                out=wb[:].rearrange("p (c h) -> p c h", c=NKC),
                in_=wbigT.rearrange("(c p) h -> p c h", p=128))
            mcS = persist.tile([2, HID], BF16)
            nc.sync.dma_start(out=mcS[:], in_=mcomb[0:2, :])
            mcU = persist.tile([9, HID], BF16)
            nc.sync.dma_start(out=mcU[:], in_=mcomb[2:11, :])
            mcC = persist.tile([1, HID], BF16)
            nc.sync.dma_start(out=mcC[:], in_=mcomb[11:12, :])
            wh = persist.tile([HID, 20], F32)
            nc.sync.dma_start(out=wh[:], in_=whead[:])

            # ---- phase A: gather + one-hot scatter matmuls
            for ch in range(N_CH):
                t0 = ch * CH_T
                cls = []
                for tab, idxd, g, ohd in ((tabS, idxS, gS, ohS),
                                          (tabA, idxA, gA, ohA),
                                          (tabB, idxB, gB, ohB)):
                    nidx = CH_T * g * 128
                    isb = gpool.tile([128, nidx // 16], mybir.dt.int16,
                                     tag=f"i{g}_{id(idxd)}")
                    nc.sync.dma_start(
                        out=isb[:], in_=idxd[:, t0 * g * 8:(t0 + CH_T) * g * 8])
                    gt = gpool.tile([128, CH_T * g, ELEM], BF16,
                                    tag=f"g{id(idxd)}")
                    nc.gpsimd.dma_gather(
                        out_ap=gt[:], in_ap=tab[:], idxs_ap=isb[:],
                        num_idxs=nidx, num_idxs_reg=nidx, elem_size=ELEM,
                        single_packet=False)
                    ohsb = gpool.tile([128, CH_T * g * 128], BF16,
                                      tag=f"oh{id(idxd)}")
                    nc.sync.dma_start(
                        out=ohsb[:],
                        in_=ohd[:, t0 * g * 128:(t0 + CH_T) * g * 128])
                    cls.append((gt, g, ohsb))
                for tl in range(CH_T):
                    t = t0 + tl
                    pS = psum.tile([2, 128], F32, space="PSUM", tag="pS")
                    pU = psum.tile([9, 128], F32, space="PSUM", tag="pU")
                    for ci, (gt, g, ohsb) in enumerate(cls):
                        for gi in range(g):
                            gcol = tl * g + gi
                            oh = ohsb[:, gcol * 128:(gcol + 1) * 128]
                            lhs = gt[:, gcol, 0:2 if ci == 0 else 9]
                            if ci == 0:
                                nc.tensor.matmul(out=pS[:], lhsT=lhs, rhs=oh,
                                                 start=(gi == 0),
                                                 stop=(gi == g - 1))
                            else:
                                nc.tensor.matmul(out=pU[:], lhsT=lhs, rhs=oh,
                                                 start=(ci == 1 and gi == 0),
                                                 stop=(ci == 2 and gi == g - 1))
                    sl = slice(t * 128, (t + 1) * 128)
                    nc.scalar.copy(out=gTs[:, sl], in_=pS[:])
                    nc.scalar.copy(out=gTu[:, sl], in_=pU[:])

            # ---- phase B: projection + KAN head per 512-node tile
            for nt in range(NT512):
                ns = slice(nt * 512, (nt + 1) * 512)
                xs = xpool.tile([128, NKC * 512], BF16, tag="xs")
                nc.sync.dma_start(
                    out=xs[:].rearrange("p (c n) -> p c n", c=NKC),
                    in_=xT[:, ns].rearrange("(c p) n -> p c n", p=128))
                pP = psum.tile([128, 512], F32, space="PSUM", tag="pP")
                for k in range(NKC):
                    nc.tensor.matmul(
                        out=pP[:], lhsT=wb[:, k * HID:(k + 1) * HID],
                        rhs=xs[:, k * 512:(k + 1) * 512],
                        start=(k == 0), stop=False)
                nc.tensor.matmul(out=pP[:], lhsT=mcS[:], rhs=gTs[:, ns],
                                 start=False, stop=False)
                nc.tensor.matmul(out=pP[:], lhsT=mcU[:], rhs=gTu[:, ns],
                                 start=False, stop=False)
                nc.tensor.matmul(out=pP[:], lhsT=mcC[:], rhs=ones2[:],
                                 start=False, stop=True)

                xt = ew.tile([128, 512], F32, tag="xt")      # clamp(h,0,2.2)
                nc.vector.tensor_scalar(out=xt[:], in0=pP[:], scalar1=0.0,
                                        scalar2=XCLAMP,
                                        op0=mybir.AluOpType.max,
                                        op1=mybir.AluOpType.min)
                sil = ew.tile([128, 512], F32, tag="sil")
                nc.scalar.activation(sil[:], pP[:],
                                     mybir.ActivationFunctionType.Silu)
                rsil = ew.tile([128, 512], F32, tag="rsil")
                nc.vector.tensor_scalar_max(rsil[:], sil[:], 0.0)
                x2 = ew.tile([128, 512], F32, tag="x2")
                nc.scalar.square(x2[:], xt[:])
                x3 = ew.tile([128, 512], F32, tag="x3")
                nc.vector.tensor_tensor(out=x3[:], in0=x2[:], in1=xt[:],
                                        op=mybir.AluOpType.mult)
                r3s = []
                for k, tk in enumerate(KNOTS):
                    r = ew.tile([128, 512], F32, tag="r")
                    nc.vector.tensor_scalar(out=r[:], in0=xt[:], scalar1=tk,
                                            scalar2=0.0,
                                            op0=mybir.AluOpType.subtract,
                                            op1=mybir.AluOpType.max)
                    r2 = ew.tile([128, 512], F32, tag="rr")
                    nc.scalar.square(r2[:], r[:])
                    r3 = ew.tile([128, 512], F32, tag=f"rrr{k}")
                    nc.vector.tensor_tensor(out=r3[:], in0=r2[:], in1=r[:],
                                            op=mybir.AluOpType.mult)
                    r3s.append(r3)
                pO = psum.tile([2, 512], F32, space="PSUM", tag="pO")
                chunks = [rsil, ones, xt, x2, x3] + r3s
                for j, ck in enumerate(chunks):
                    nc.tensor.matmul(out=pO[:], lhsT=wh[:, 2 * j:2 * j + 2],
                                     rhs=ck[:], start=(j == 0), stop=(j == 9))
                ot = ew.tile([2, 512], F32, tag="ot")
                nc.vector.tensor_copy(out=ot[:], in_=pO[:])
                nc.sync.dma_start(out=outT[:, ns], in_=ot[:])

    nc.compile()
    return nc


# ----------------------------------------------------------------- entry point
def kernel(**inp):
    inp = {k: np.asarray(v) for k, v in inp.items()}
    wbigT, mcomb, head = _fold_weights(inp)
    eprep = _prep_edges(inp)
    (idxS, dstS, gS), (idxA, dstA, gA), (idxB, dstB, gB) = eprep

    key = (gS, gA, gB)
    if key not in _CACHE:
        _CACHE[key] = _build(gS, gA, gB)
    nc = _CACHE[key]

    tabS = np.zeros((NS + 1, ELEM), BF)
    tabS[:NS, 0] = inp["x_sender"][:, 0].astype(BF)
    tabS[:NS, 1] = 1
    tabA = np.zeros((URL_SPLIT + 1, ELEM), BF)
    tabA[:URL_SPLIT, 0:8] = inp["x_url"][:URL_SPLIT].astype(BF)
    tabA[:URL_SPLIT, 8] = 1
    tabB = np.zeros((NU - URL_SPLIT + 1, ELEM), BF)
    tabB[: NU - URL_SPLIT, 0:8] = inp["x_url"][URL_SPLIT:].astype(BF)
    tabB[: NU - URL_SPLIT, 8] = 1

    in_maps = []
    for c in range(N_CORES):
        xsh = np.zeros((KIN, NP), BF)
        xsh[:, :NSH] = inp["x_email"][c * NSH:(c + 1) * NSH].T.astype(BF)
        in_maps.append({
            "xT": xsh, "tabS": tabS, "tabA": tabA, "tabB": tabB,
            "idxS": idxS[c], "idxA": idxA[c], "idxB": idxB[c],
            "ohS": dstS[c], "ohA": dstA[c], "ohB": dstB[c],
            "wbigT": wbigT.astype(BF), "mcomb": mcomb.astype(BF), "whead": head,
        })

    global _LAST_RESULT
    trace = os.environ.get("KERNEL_TRACE", "0") == "1"
    res = run_bass_kernel_spmd(nc, in_maps, core_ids=list(range(N_CORES)),
                               trace=trace)
    _LAST_RESULT = res
    out = np.empty((NE, 2), np.float32)
    for c in range(N_CORES):
        out[c * NSH:(c + 1) * NSH] = res.results[c]["outT"][:, :NSH].T
    return out


# revision 10
# speedup vs baseline: 2.2367x; 1.1100x over previous
"""Trainium2 Bass kernel for nn_HKANGNN (hetero GraphConv + KAN head).

Math (only the email-node output path matters):
  e    = x_email @ w_email.T + b_email
  agg_se[n] = sum_{se edges -> n} (x_sender[src] @ w_sender.T + b_sender)
  agg_ue[n] = sum_{ue edges -> n} (x_url[src]    @ w_url.T    + b_url)
  out_e = agg_se @ w_rel_se.T + b_rel_se + agg_ue @ w_rel_ue.T + b_rel_ue
        + e @ (w_root_se + w_root_ue).T
  h = relu(out_e);  out = silu(h) @ base_w.T + einsum(b_splines(h), spline_w)

Device strategy (8 cores, email nodes sharded 12500/core, padded to 12800):
  * linearity lets the per-edge features be the RAW source features plus a
    count column; the tiny projection matrices fold into Mcomb on host.
  * segment-sum = one-hot matmuls accumulated in PSUM per 128-dst tile;
    per-edge rows fetched with dma_gather (3 source classes so int16 idx fit).
  * projection: out_e.T[h, n] accumulated in PSUM over 6 K-chunks of
    (Wrootsum@w_email).T as stationary weights (bf16).
  * KAN head: h>=0 and all bases vanish for h>=2.2, so with x = clamp(h,0,2.2)
    spline(h) == q0+q1 x+q2 x^2+q3 x^3 + sum_k W'_k relu(x-t_k)^3 (t_k=.2..1.8)
    -> 10 extra matmul K-chunks ([silu,1,x,x^2,x^3,R6^3..R10^3]) into a [2,512]
    PSUM per 512-node tile.
"""

import os
import numpy as np
import ml_dtypes

import concourse.bass as bass
import concourse.mybir as mybir
import concourse.tile as tile
from concourse import bacc
from concourse.bass_utils import run_bass_kernel_spmd

F32 = mybir.dt.float32
BF16 = mybir.dt.bfloat16
BF = ml_dtypes.bfloat16

N_CORES = 8
HID = 128
NE, NS, NU = 100000, 30000, 50000
NSH = NE // N_CORES          # 12500 real nodes per core
NP = 12800                   # padded (25 x 512 node tiles, 100 x 128 dst tiles)
NT128 = NP // 128            # 100 dst tiles
NT512 = NP // 512            # 25 node tiles
KIN = 768
NKC = KIN // 128             # 6 projection K-chunks
URL_SPLIT = 25600            # url class A rows [0,25600), B rows [25600,50000)
ELEM = 128                   # gather row: 128 bf16 = 256 B
CH_T = 10                    # dst tiles per gather chunk
N_CH = NT128 // CH_T         # 10 chunks
KNOTS = (0.2, 0.6, 1.0, 1.4, 1.8)
XCLAMP = 2.2

_LAST_RESULT = None
_CACHE = {}


# ----------------------------------------------------------------- host folds
def _head_weights(base_w, spline_w):
    """[128, 20] f32: lhsT ([d,2]) per head chunk, order
    [silu, ones, x, x^2, x^3, R(.2)^3, R(.6)^3, R(1.0)^3, R(1.4)^3, R(1.8)^3]."""
    c = np.array([1.0, -4.0, 6.0, -4.0, 1.0], np.float64)
    h = 0.4
    scale = 1.0 / (6.0 * h ** 3)
    O, D, B = spline_w.shape                      # [2, 128, 8]
    wp = np.zeros((O, D, 11), np.float64)         # W'[o,d,m], m=0..10
    for m in range(11):
        for j in range(5):
            b = m - j
            if 0 <= b < B:
                wp[:, :, m] += spline_w[:, :, b].astype(np.float64) * c[j] * scale
    t = np.arange(11) * h - 2.2                   # knot m at t_m
    q = np.zeros((4, O, D), np.float64)           # poly coeffs from m=0..5
    for m in range(6):
        q[0] += -t[m] ** 3 * wp[:, :, m]
        q[1] += 3 * t[m] ** 2 * wp[:, :, m]
        q[2] += -3 * t[m] * wp[:, :, m]
        q[3] += wp[:, :, m]
    head = np.zeros((D, 20), np.float64)
    head[:, 0:2] = base_w.T                       # silu chunk
    for j in range(4):                            # ones, x, x^2, x^3
        head[:, 2 * (1 + j):2 * (1 + j) + 2] = q[j].T
    for k in range(5):                            # relu^3 knots m=6..10
        head[:, 2 * (5 + k):2 * (5 + k) + 2] = wp[:, :, 6 + k].T
    return head.astype(np.float32)


def _fold_weights(inp):
    wrs = inp["w_root_se"] + inp["w_root_ue"]
    wbigT = (wrs @ inp["w_email"]).T.copy()                     # [768, 128]
    mcomb = np.zeros((12, 128), np.float32)
    mcomb[0] = inp["w_rel_se"] @ inp["w_sender"][:, 0]
    mcomb[1] = inp["w_rel_se"] @ inp["b_sender"]
    mcomb[2:10] = (inp["w_rel_ue"] @ inp["w_url"]).T
    mcomb[10] = inp["w_rel_ue"] @ inp["b_url"]
    mcomb[11] = inp["b_rel_se"] + inp["b_rel_ue"] + wrs @ inp["b_email"]
    head = _head_weights(inp["base_w"], inp["spline_w"])
    return wbigT, mcomb, head


def _wrap_idx16(flat):
    """int16 slot list -> [128, n/16] wrapped in 16 partitions, tiled to 128."""
    n = flat.shape[0]
    a = flat.astype(np.int16).reshape(n // 16, 16).T            # [16, n/16]
    return np.tile(a, (8, 1))


def _prep_edges(inp):
    """Per-core per-class slot arrays (idx into class tables + local dst)."""
    cls_edges = []
    # (src_in_class_table, dst_email) per class
    cls_edges.append((inp["se_src"], inp["se_dst"]))                     # S
    ua = inp["ue_src"] < URL_SPLIT
    cls_edges.append((inp["ue_src"][ua], inp["ue_dst"][ua]))             # A
    cls_edges.append((inp["ue_src"][~ua] - URL_SPLIT, inp["ue_dst"][~ua]))  # B
    zrow = (NS, URL_SPLIT, NU - URL_SPLIT)                               # zero-row ids
    percls = []
    for ci, (src, dst) in enumerate(cls_edges):
        per_core = []
        gmax = 1
        for c in range(N_CORES):
            sel = (dst >= c * NSH) & (dst < (c + 1) * NSH)
            s, d = src[sel], dst[sel] - c * NSH
            order = np.argsort(d, kind="stable")
            s, d = s[order], d[order]
            tlist = []
            for t in range(NT128):
                m = (d >= t * 128) & (d < (t + 1) * 128)
                tlist.append((s[m], d[m] - t * 128))
                gmax = max(gmax, (len(tlist[-1][0]) + 127) // 128)
            per_core.append(tlist)
        percls.append((per_core, gmax, zrow[ci]))
    out = []
    for per_core, gmax, zr in percls:
        idxs, dsts = [], []
        for c in range(N_CORES):
            slots = np.full((NT128, gmax * 128), zr, np.int32)
            dloc = np.zeros((NT128, gmax * 128), np.float32)
            for t, (s, d) in enumerate(per_core[c]):
                slots[t, : len(s)] = s
                dloc[t, : len(s)] = d
            # slot j within a group of 128 -> partition j%128; groups tile-major
            flat = slots.reshape(-1)                             # [100*gmax*128]
            idxs.append(_wrap_idx16(flat))
            oh = (dloc.reshape(NT128 * gmax, 128).T[:, :, None]
                  == np.arange(128)[None, None, :]).astype(BF)
            dsts.append(oh.reshape(128, NT128 * gmax * 128))   # [128, ngrp*128]
        out.append((np.stack(idxs), np.stack(dsts), gmax))
    return out  # [(idx16 [8,128,*], dst [8,128,100*g], g)] * 3


# ----------------------------------------------------------------- device build
def _build(gS, gA, gB):
    nc = bacc.Bacc("TRN2", target_bir_lowering=False, debug=False,
                   num_devices=N_CORES)
    dt = lambda n, s, d, k: nc.dram_tensor(n, s, d, kind=k).ap()
    xT = dt("xT", [KIN, NP], BF16, "ExternalInput")
    tabS = dt("tabS", [NS + 1, ELEM], BF16, "ExternalInput")
    tabA = dt("tabA", [URL_SPLIT + 1, ELEM], BF16, "ExternalInput")
    tabB = dt("tabB", [NU - URL_SPLIT + 1, ELEM], BF16, "ExternalInput")
    idxS = dt("idxS", [128, NT128 * gS * 8], mybir.dt.int16, "ExternalInput")
    idxA = dt("idxA", [128, NT128 * gA * 8], mybir.dt.int16, "ExternalInput")
    idxB = dt("idxB", [128, NT128 * gB * 8], mybir.dt.int16, "ExternalInput")
    ohS = dt("ohS", [128, NT128 * gS * 128], BF16, "ExternalInput")
    ohA = dt("ohA", [128, NT128 * gA * 128], BF16, "ExternalInput")
    ohB = dt("ohB", [128, NT128 * gB * 128], BF16, "ExternalInput")
    wbigT = dt("wbigT", [KIN, HID], BF16, "ExternalInput")
    mcomb = dt("mcomb", [12, HID], BF16, "ExternalInput")
    whead = dt("whead", [HID, 20], F32, "ExternalInput")
    outT = dt("outT", [2, NP], F32, "ExternalOutput")

    with tile.TileContext(nc) as tc:
        import contextlib
        with contextlib.ExitStack() as ctx:
            persist = ctx.enter_context(tc.tile_pool(name="persist", bufs=1))
            gpool = ctx.enter_context(tc.tile_pool(name="gath", bufs=2))
            xpool = ctx.enter_context(tc.tile_pool(name="x", bufs=2))
            ew = ctx.enter_context(tc.tile_pool(name="ew", bufs=2))
            psum = ctx.enter_context(tc.tile_pool(name="ps", bufs=2, space="PSUM"))

            # ---- persistent small tensors
            ones = persist.tile([128, 512], F32)
            nc.gpsimd.memset(ones[:], 1.0)
            gTs = persist.tile([2, NP], BF16)
            gTu = persist.tile([9, NP], BF16)
            ones2 = persist.tile([1, 512], BF16)
            nc.gpsimd.memset(ones2[:], 1.0)
            wb = persist.tile([128, NKC * HID], BF16)
            nc.sync.dma_start(
                out=wb[:].rearrange("p (c h) -> p c h", c=NKC),
                in_=wbigT.rearrange("(c p) h -> p c h", p=128))
            mcS = persist.tile([2, HID], BF16)
            nc.sync.dma_start(out=mcS[:], in_=mcomb[0:2, :])
            mcU = persist.tile([9, HID], BF16)
            nc.sync.dma_start(out=mcU[:], in_=mcomb[2:11, :])
            mcC = persist.tile([1, HID], BF16)
            nc.sync.dma_start(out=mcC[:], in_=mcomb[11:12, :])
            wh = persist.tile([HID, 20], F32)
            nc.sync.dma_start(out=wh[:], in_=whead[:])

            # ---- phase B emitter (interleaved with phase A chunks)
            def phase_b(nt):
                ns = slice(nt * 512, (nt + 1) * 512)
                xs = xpool.tile([128, NKC * 512], BF16, tag="xs")
                nc.sync.dma_start(
                    out=xs[:].rearrange("p (c n) -> p c n", c=NKC),
                    in_=xT[:, ns].rearrange("(c p) n -> p c n", p=128))
                pP = psum.tile([128, 512], F32, space="PSUM", tag="pP")
                for k in range(NKC):
                    nc.tensor.matmul(
                        out=pP[:], lhsT=wb[:, k * HID:(k + 1) * HID],
                        rhs=xs[:, k * 512:(k + 1) * 512],
                        start=(k == 0), stop=False)
                nc.tensor.matmul(out=pP[:], lhsT=mcS[:], rhs=gTs[:, ns],
                                 start=False, stop=False)
                nc.tensor.matmul(out=pP[:], lhsT=mcU[:], rhs=gTu[:, ns],
                                 start=False, stop=False)
                nc.tensor.matmul(out=pP[:], lhsT=mcC[:], rhs=ones2[:],
                                 start=False, stop=True)

                xt = ew.tile([128, 512], F32, tag="xt")      # clamp(h,0,2.2)
                nc.vector.tensor_scalar(out=xt[:], in0=pP[:], scalar1=0.0,
                                        scalar2=XCLAMP,
                                        op0=mybir.AluOpType.max,
                                        op1=mybir.AluOpType.min)
                sil = ew.tile([128, 512], F32, tag="sil")
                nc.scalar.activation(sil[:], pP[:],
                                     mybir.ActivationFunctionType.Silu)
                rsil = ew.tile([128, 512], F32, tag="rsil")
                nc.vector.tensor_scalar_max(rsil[:], sil[:], 0.0)
                x2 = ew.tile([128, 512], F32, tag="x2")
                nc.scalar.square(x2[:], xt[:])
                x3 = ew.tile([128, 512], F32, tag="x3")
                nc.vector.tensor_tensor(out=x3[:], in0=x2[:], in1=xt[:],
                                        op=mybir.AluOpType.mult)
                r3s = []
                for k, tk in enumerate(KNOTS):
                    r = ew.tile([128, 512], F32, tag="r")
                    nc.vector.tensor_scalar(out=r[:], in0=xt[:], scalar1=tk,
                                            scalar2=0.0,
                                            op0=mybir.AluOpType.subtract,
                                            op1=mybir.AluOpType.max)
                    r2 = ew.tile([128, 512], F32, tag="rr")
                    nc.scalar.square(r2[:], r[:])
                    r3 = ew.tile([128, 512], F32, tag=f"rrr{k}")
                    nc.vector.tensor_tensor(out=r3[:], in0=r2[:], in1=r[:],
                                            op=mybir.AluOpType.mult)
                    r3s.append(r3)
                pO = psum.tile([2, 512], F32, space="PSUM", tag="pO")
                chunks = [rsil, ones, xt, x2, x3] + r3s
                for j, ck in enumerate(chunks):
                    nc.tensor.matmul(out=pO[:], lhsT=wh[:, 2 * j:2 * j + 2],
                                     rhs=ck[:], start=(j == 0), stop=(j == 9))
                ot = ew.tile([2, 512], F32, tag="ot")
                nc.vector.tensor_copy(out=ot[:], in_=pO[:])
                nc.sync.dma_start(out=outT[:, ns], in_=ot[:])

            # ---- phase A: gather + one-hot scatter matmuls
            next_nt = [0]
            for ch in range(N_CH):
                t0 = ch * CH_T
                cls = []
                for tab, idxd, g, ohd in ((tabS, idxS, gS, ohS),
                                          (tabA, idxA, gA, ohA),
                                          (tabB, idxB, gB, ohB)):
                    nidx = CH_T * g * 128
                    isb = gpool.tile([128, nidx // 16], mybir.dt.int16,
                                     tag=f"i{g}_{id(idxd)}")
                    nc.sync.dma_start(
                        out=isb[:], in_=idxd[:, t0 * g * 8:(t0 + CH_T) * g * 8])
                    gt = gpool.tile([128, CH_T * g, ELEM], BF16,
                                    tag=f"g{id(idxd)}")
                    nc.gpsimd.dma_gather(
                        out_ap=gt[:], in_ap=tab[:], idxs_ap=isb[:],
                        num_idxs=nidx, num_idxs_reg=nidx, elem_size=ELEM,
                        single_packet=False)
                    ohsb = gpool.tile([128, CH_T * g * 128], BF16,
                                      tag=f"oh{id(idxd)}")
                    nc.sync.dma_start(
                        out=ohsb[:],
                        in_=ohd[:, t0 * g * 128:(t0 + CH_T) * g * 128])
                    cls.append((gt, g, ohsb))
                for tl in range(CH_T):
                    t = t0 + tl
                    pS = psum.tile([2, 128], F32, space="PSUM", tag="pS")
                    pU = psum.tile([9, 128], F32, space="PSUM", tag="pU")
                    for ci, (gt, g, ohsb) in enumerate(cls):
                        for gi in range(g):
                            gcol = tl * g + gi
                            oh = ohsb[:, gcol * 128:(gcol + 1) * 128]
                            lhs = gt[:, gcol, 0:2 if ci == 0 else 9]
                            if ci == 0:
                                nc.tensor.matmul(out=pS[:], lhsT=lhs, rhs=oh,
                                                 start=(gi == 0),
                                                 stop=(gi == g - 1))
                            else:
                                nc.tensor.matmul(out=pU[:], lhsT=lhs, rhs=oh,
                                                 start=(ci == 1 and gi == 0),
                                                 stop=(ci == 2 and gi == g - 1))
                    sl = slice(t * 128, (t + 1) * 128)
                    nc.scalar.copy(out=gTs[:, sl], in_=pS[:])
                    nc.scalar.copy(out=gTu[:, sl], in_=pU[:])
                ready = ((ch + 1) * CH_T) // 4          # node tiles with gT done
                while next_nt[0] < (ready if ch < N_CH - 1 else NT512):
                    phase_b(next_nt[0])
                    next_nt[0] += 1


    nc.compile()
    return nc


# ----------------------------------------------------------------- entry point
def kernel(**inp):
    inp = {k: np.asarray(v) for k, v in inp.items()}
    wbigT, mcomb, head = _fold_weights(inp)
    eprep = _prep_edges(inp)
    (idxS, dstS, gS), (idxA, dstA, gA), (idxB, dstB, gB) = eprep

    key = (gS, gA, gB)
    if key not in _CACHE:
        _CACHE[key] = _build(gS, gA, gB)
    nc = _CACHE[key]

    tabS = np.zeros((NS + 1, ELEM), BF)
    tabS[:NS, 0] = inp["x_sender"][:, 0].astype(BF)
    tabS[:NS, 1] = 1
    tabA = np.zeros((URL_SPLIT + 1, ELEM), BF)
    tabA[:URL_SPLIT, 0:8] = inp["x_url"][:URL_SPLIT].astype(BF)
    tabA[:URL_SPLIT, 8] = 1
    tabB = np.zeros((NU - URL_SPLIT + 1, ELEM), BF)
    tabB[: NU - URL_SPLIT, 0:8] = inp["x_url"][URL_SPLIT:].astype(BF)
    tabB[: NU - URL_SPLIT, 8] = 1

    in_maps = []
    for c in range(N_CORES):
        xsh = np.zeros((KIN, NP), BF)
        xsh[:, :NSH] = inp["x_email"][c * NSH:(c + 1) * NSH].T.astype(BF)
        in_maps.append({
            "xT": xsh, "tabS": tabS, "tabA": tabA, "tabB": tabB,
            "idxS": idxS[c], "idxA": idxA[c], "idxB": idxB[c],
            "ohS": dstS[c], "ohA": dstA[c], "ohB": dstB[c],
            "wbigT": wbigT.astype(BF), "mcomb": mcomb.astype(BF), "whead": head,
        })

    global _LAST_RESULT
    trace = os.environ.get("KERNEL_TRACE", "0") == "1"
    res = run_bass_kernel_spmd(nc, in_maps, core_ids=list(range(N_CORES)),
                               trace=trace)
    _LAST_RESULT = res
    out = np.empty((NE, 2), np.float32)
    for c in range(N_CORES):
        out[c * NSH:(c + 1) * NSH] = res.results[c]["outT"][:, :NSH].T
    return out
